# revision 5
# baseline (speedup 1.0000x reference)
"""MultiHeadAttention (B=2, S=2048, D=1024, H=16, depth=64) on 8 trn2 cores. v3

Sharding: core c -> batch b=c//4, head-group g=c%4 (heads 4g..4g+3).

v3 = v2 software-pipelined ACROSS repeats: the ACT exp stream (the hard
~147us/rep floor) never drains. Projections for rep r+1 and the output
projection of rep r are chopped into "filler units" popped into the
ACT-bound bubbles of rep r's attention g-loops. All long-lived tiles are
double-buffered (bufs=2) so consecutive reps ping-pong buffers instead of
serializing on write-after-read hazards.

Per-core device program highlights (see v2 docstring for the math):
  - bk dropped exactly (softmax shift-invariance), bv folded into bo on host.
  - V projection computed directly seq-major into vI [128, 4, 65] with an
    all-ones column 64 per head => attn@V also emits softmax denominators.
  - scores: two heads packed in the PE array via tile_position (0,0)/(64,0).
  - normalization: 2 reciprocals -> one rank-2 PE broadcast matmul ->
    2 fused DVE multiplies, deferred into the next loop to hide latency.
  - PSUM budget (8 banks): sup 2x[128,2,512] (4) + ctx 2x[65,512] (2) +
    aux 2x[128,512] (2) shared by projection and output-projection units.
"""

import numpy as np
import ml_dtypes

B, S, D = 2, 2048, 1024
FG = 256  # features per core (4 heads x 64)

_compiled = None


def _build_program(repeat=1, do_proj=True, do_attn=True, do_exp=True,
                   do_xdma=True):
    import concourse.bass as bass  # noqa: F401
    import concourse.tile as tile
    from concourse import bacc, mybir

    f32 = mybir.dt.float32
    f32r = mybir.dt.float32r
    bf16 = mybir.dt.bfloat16
    EXP = mybir.ActivationFunctionType.Exp
    MULT = mybir.AluOpType.mult

    nc = bacc.Bacc("TRN2", target_bir_lowering=False, debug=False)

    xq_d = nc.dram_tensor("xq", [D, S], bf16, kind="ExternalInput")
    xk_d = nc.dram_tensor("xk", [D, S], bf16, kind="ExternalInput")
    xv_d = nc.dram_tensor("xv", [D, S], bf16, kind="ExternalInput")
    wq_d = nc.dram_tensor("wq", [D, FG], bf16, kind="ExternalInput")
    wk_d = nc.dram_tensor("wk", [D, FG], bf16, kind="ExternalInput")
    wv_d = nc.dram_tensor("wv", [D, FG], bf16, kind="ExternalInput")
    wo_d = nc.dram_tensor("wo", [FG, D], bf16, kind="ExternalInput")
    bq_d = nc.dram_tensor("bq", [FG, 1], f32, kind="ExternalInput")
    out_d = nc.dram_tensor("out", [D, S], bf16, kind="ExternalOutput")

    with tile.TileContext(nc) as tc:
      with tc.tile_pool(name="const", bufs=1) as cpool:

        # ---- per-rep long-lived tiles (bufs=2 => reps ping-pong) ----------
        def rep_tiles(r):
            t = {}
            t["zbias"] = cpool.tile([128, 1], f32, tag="zbias", bufs=2,
                                    name="zbias")
            nc.gpsimd.memset(t["zbias"][:], 0.0)
            onesf = cpool.tile([1, 64], f32, tag="onesf", bufs=2,
                               name="onesf")
            nc.gpsimd.memset(onesf[:], 1.0)
            t["ones_r"] = cpool.tile([1, 64], f32r, tag="ones_r", bufs=2,
                                     name="ones_r")
            nc.vector.tensor_copy(t["ones_r"][:], onesf[:])
            o41f = cpool.tile([128, 4, 1], f32, tag="o41f", bufs=2,
                              name="o41f")
            nc.gpsimd.memset(o41f[:], 1.0)

            t["w"] = {}
            for nm, dd in (("wq", wq_d), ("wk", wk_d), ("wv", wv_d)):
                for kk in range(8):
                    w = cpool.tile([128, FG], bf16, tag=f"{nm}{kk}", bufs=1,
                                   name=f"{nm}{kk}")
                    nc.sync.dma_start(w[:], dd.ap()[128 * kk:128 * (kk + 1), :])
                    t["w"][(nm, kk)] = w
            t["bq"] = []
            for pch in range(2):
                bqt = cpool.tile([128, 1], f32, tag=f"bq{pch}", bufs=1,
                                 name=f"bq{pch}")
                nc.sync.dma_start(bqt[:], bq_d.ap()[128 * pch:128 * (pch + 1), :])
                t["bq"].append(bqt)
            t["wo"] = []
            for kk2 in range(2):
                wot = cpool.tile([128, D], bf16, tag=f"wo{kk2}", bufs=2,
                                 name=f"wo{kk2}")
                nc.sync.dma_start(wot[:], wo_d.ap()[128 * kk2:128 * (kk2 + 1), :])
                t["wo"].append(wot)

            t["xs"] = {}
            for nm, dd in (("xk", xk_d), ("xv", xv_d), ("xq", xq_d)):
                for kk in range(8):
                    xs = cpool.tile([128, S], bf16, tag=f"{nm}s{kk}", bufs=1,
                                    name=f"{nm}s{kk}")
                    if do_xdma:
                        nc.sync.dma_start(xs[:],
                                          dd.ap()[128 * kk:128 * (kk + 1), :])
                    else:
                        nc.gpsimd.memset(xs[:], 0.0)
                    t["xs"][(nm, kk)] = xs

            t["qT"] = [cpool.tile([128, S], bf16, tag=f"qT{p}", bufs=2,
                                  name=f"qT{p}") for p in range(2)]
            t["kT"] = [cpool.tile([128, S], bf16, tag=f"kT{p}", bufs=2,
                                  name=f"kT{p}") for p in range(2)]
            t["vI"] = [cpool.tile([128, 4, 65], bf16, tag=f"vI{sc}", bufs=2,
                                  name=f"vI{sc}") for sc in range(16)]
            for sc in range(16):
                nc.vector.tensor_copy(t["vI"][sc][:, :, 64:65], o41f[:])
            t["ctxN"] = [cpool.tile([128, S], bf16, tag=f"ctxN{p}", bufs=2,
                                    name=f"ctxN{p}") for p in range(2)]
            if not do_proj:
                # probe mode: zero-fill qT/kT/vI
                zst = cpool.tile([128, S], f32, tag="zst", bufs=1, name="zst")
                nc.gpsimd.memset(zst[:], 0.0)
                for p in range(2):
                    nc.gpsimd.tensor_copy(t["qT"][p][:], zst[:])
                    nc.gpsimd.tensor_copy(t["kT"][p][:], zst[:])
                for sc in range(16):
                    nc.vector.tensor_copy(t["vI"][sc][:, :, 0:64],
                                          zst[:, 0:256])
            return t

        # ---- projection filler units (run inside the previous rep) --------
        def proj_units(t, aux_tile):
            """Yield closures; each does ~8 matmuls + 1 stage copy."""
            units = []

            def k_unit(pch, qc, wname, outT, bias):
                def run():
                    ps = aux_tile()
                    for kk in range(8):
                        nc.tensor.matmul(
                            ps[:],
                            t["w"][(wname, kk)][:, 128 * pch:128 * (pch + 1)],
                            t["xs"][({"wk": "xk", "wq": "xq"}[wname], kk)][
                                :, 512 * qc:512 * (qc + 1)],
                            start=(kk == 0), stop=(kk == 7))
                    dst = outT[pch][:, 512 * qc:512 * (qc + 1)]
                    if bias is None:
                        nc.vector.tensor_copy(dst, ps[:])
                    else:
                        nc.vector.tensor_scalar_add(dst, ps[:],
                                                    t["bq"][pch][:, :])
                return run

            def v_unit(sc):
                def run():
                    ps = aux_tile()
                    for kk in range(8):
                        nc.tensor.matmul(
                            ps[:, 0:256],
                            t["xs"][("xv", kk)][:, 128 * sc:128 * (sc + 1)],
                            t["w"][("wv", kk)][:, :],
                            start=(kk == 0), stop=(kk == 7))
                    nc.vector.tensor_copy(
                        t["vI"][sc][:, :, 0:64],
                        ps[:, 0:256].rearrange("p (h d) -> p h d", h=4))
                return run

            for pch in range(2):
                for qc in range(4):
                    units.append(k_unit(pch, qc, "wk", t["kT"], None))
            for sc in range(16):
                units.append(v_unit(sc))
            for qc in range(4):
                for pch in range(2):
                    units.append(k_unit(pch, qc, "wq", t["qT"], "bq"))
            return units

        with tc.tile_pool(name="scp", bufs=1, space="PSUM") as scp, \
             tc.tile_pool(name="cxp", bufs=1, space="PSUM") as cxp, \
             tc.tile_pool(name="aux", bufs=2, space="PSUM") as aux, \
             tc.tile_pool(name="exp", bufs=6) as expool, \
             tc.tile_pool(name="rcp", bufs=2) as rcpool, \
             tc.tile_pool(name="csp", bufs=2) as cspool, \
             tc.tile_pool(name="obp", bufs=2) as obpool:

            def aux_tile():
                return aux.tile([128, 512], f32, name="aux", bufs=2)

            pending_norm = [None]
            fillers = []

            def flush_norm():
                if pending_norm[0] is not None:
                    pending_norm[0]()
                    pending_norm[0] = None

            def outproj_unit(t, qj, m):
                def run():
                    op = aux_tile()
                    for kk2 in range(2):
                        nc.tensor.matmul(
                            op[:],
                            t["wo"][kk2][:, 128 * m:128 * (m + 1)],
                            t["ctxN"][kk2][:, 512 * qj:512 * (qj + 1)],
                            start=(kk2 == 0), stop=(kk2 == 1))
                    ob = obpool.tile([128, 512], bf16, name="ob", bufs=4)
                    nc.vector.tensor_copy(ob[:], op[:])
                    nc.sync.dma_start(
                        out_d.ap()[128 * m:128 * (m + 1),
                                   512 * qj:512 * (qj + 1)],
                        ob[:])
                return run

            def attention(t, final):
                for qj in range(4):
                    for hp in range(2):
                        pch = hp
                        ctxs = [cxp.tile([65, 512], f32, name=f"ctx{hh}",
                                         bufs=1) for hh in range(2)]
                        pend = None
                        for g in range(8):
                            cur = []
                            for hh in range(2):
                                off = 64 * hh
                                sup = scp.tile([128, 2, 512], f32, name="sup",
                                               bufs=2)
                                for j in range(2):
                                    ki = 2 * g + j
                                    nc.tensor.matmul(
                                        sup[:, j, :],
                                        t["kT"][pch][off:off + 64,
                                                     128 * ki:128 * (ki + 1)],
                                        t["qT"][pch][off:off + 64,
                                                     512 * qj:512 * (qj + 1)],
                                        start=True, stop=True,
                                        tile_position=(off, 0))
                                if do_exp:
                                    ex = expool.tile([128, 2, 512], bf16,
                                                     name="ex", bufs=4)
                                    nc.scalar.activation(ex[:], sup[:], EXP,
                                                         bias=t["zbias"][:],
                                                         scale=0.125)
                                    cur.append(ex)
                                else:
                                    cur.append(None)
                            if g == 1:
                                flush_norm()
                            elif g >= 2 and fillers:
                                fillers.pop(0)()
                                if len(fillers) > 40 and fillers:
                                    fillers.pop(0)()
                            if pend is not None:
                                for hh in range(2):
                                    for j in range(2):
                                        pk = 2 * (g - 1) + j
                                        mv = (pend[hh][:, j, :] if do_exp else
                                              t["qT"][pch][:, 512 * qj:
                                                           512 * (qj + 1)])
                                        nc.tensor.matmul(
                                            ctxs[hh][:],
                                            t["vI"][pk][:, 2 * pch + hh, :],
                                            mv,
                                            start=(pk == 0), stop=False)
                            pend = cur
                        for hh in range(2):
                            for j in range(2):
                                pk = 14 + j
                                mv = (pend[hh][:, j, :] if do_exp else
                                      t["qT"][pch][:, 512 * qj:512 * (qj + 1)])
                                nc.tensor.matmul(
                                    ctxs[hh][:], t["vI"][pk][:, 2 * pch + hh, :],
                                    mv, start=False, stop=(pk == 15))
                        rcs = []
                        with nc.allow_low_precision(
                                reason="f32r for PE broadcast"):
                            for hh in range(2):
                                rc = rcpool.tile([1, 512], f32r, name="rc",
                                                 bufs=4)
                                nc.vector.reciprocal(rc[:],
                                                     ctxs[hh][64:65, :])
                                rcs.append(rc)

                        def norm(t=t, pch=pch, qj=qj, ctxs=ctxs, rcs=rcs):
                            # only one PSUM operand allowed per DVE op: stage
                            # ctx to SBUF, then multiply by the PSUM broadcast
                            for hh in range(2):
                                off = 64 * hh
                                cs = cspool.tile([64, 512], bf16, name="cs",
                                                 bufs=4)
                                nc.vector.tensor_copy(cs[:], ctxs[hh][0:64, :])
                                bc = aux_tile()
                                nc.tensor.matmul(bc[0:64, :],
                                                 t["ones_r"][:, :],
                                                 rcs[hh][:, :],
                                                 start=True, stop=True)
                                nc.vector.tensor_tensor(
                                    t["ctxN"][pch][off:off + 64,
                                                   512 * qj:512 * (qj + 1)],
                                    cs[:],
                                    bc[0:64, :], MULT)
                        pending_norm[0] = norm
                    units = [outproj_unit(t, qj, m) for m in range(8)]
                    if qj < 3 or not final:
                        fillers.extend(units)
                    else:
                        flush_norm()
                        for u in fillers + units:
                            u()
                        del fillers[:]

            # ================== emission ==================
            t_cur = rep_tiles(0)
            if do_proj:
                for u in proj_units(t_cur, aux_tile):
                    u()
            for r in range(repeat):
                if not do_attn:
                    del fillers[:]
                    pending_norm[0] = None
                    break
                if r + 1 < repeat:
                    t_nxt = rep_tiles(r + 1)
                    if do_proj:
                        fillers.extend(proj_units(t_nxt, aux_tile))
                else:
                    t_nxt = None
                attention(t_cur, final=(r + 1 == repeat))
                t_cur = t_nxt

    nc.compile()
    return nc


def _make_in_maps(q, k, v, wq, bq, wk, bk, wv, bv, wo):
    bf = ml_dtypes.bfloat16
    in_maps = []
    for c in range(8):
        b, g = divmod(c, 4)
        fs = slice(FG * g, FG * (g + 1))
        in_maps.append({
            "xq": np.ascontiguousarray(q[b].T.astype(bf)),
            "xk": np.ascontiguousarray(k[b].T.astype(bf)),
            "xv": np.ascontiguousarray(v[b].T.astype(bf)),
            "wq": np.ascontiguousarray(wq[fs, :].T.astype(bf)),
            "wk": np.ascontiguousarray(wk[fs, :].T.astype(bf)),
            "wv": np.ascontiguousarray(wv[fs, :].T.astype(bf)),
            "wo": np.ascontiguousarray(wo[:, fs].T.astype(bf)),
            "bq": np.ascontiguousarray(bq[fs].reshape(FG, 1).astype(np.float32)),
        })
    return in_maps


def kernel(q, k, v, wq, bq, wk, bk, wv, bv, wo, bo):
    from concourse.bass_utils import run_bass_kernel_spmd

    global _compiled
    if _compiled is None:
        _compiled = _build_program()
    nc = _compiled

    args = [np.asarray(a, dtype=np.float32)
            for a in (q, k, v, wq, bq, wk, bk, wv, bv, wo)]
    bo = np.asarray(bo, dtype=np.float32)
    wv_f = args[7]
    bv_f = args[8]
    wo_f = args[9]
    # bv folded on host: out += wo @ bv (constant over q). bk dropped exactly
    # (softmax shift-invariance).
    bo_eff = bo + wo_f @ bv_f
    in_maps = _make_in_maps(*args)
    res = run_bass_kernel_spmd(nc, in_maps, core_ids=list(range(8)))
    outs = [np.asarray(res.results[c]["out"]).astype(np.float32)
            for c in range(8)]
    full = []
    for b in range(B):
        acc = outs[4 * b] + outs[4 * b + 1] + outs[4 * b + 2] + outs[4 * b + 3]
        full.append(acc.T + bo_eff[None, :])
    return np.stack(full).astype(np.float32)
